# revision 2
# baseline (speedup 1.0000x reference)
"""EpsSupInfoNCE loss on 8 Trainium2 NeuronCores.

Math (reference): logits = (E @ E.T)/temp;  same[i,j] = (label_i == label_j)
  S_j   = sum_i exp(logits[i,j]) * (1 - same[i,j])     (masked column sums)
  ce_ij = log(exp(-eps) + S_j * exp(-logits[i,j]))     for same-label i != j
  loss  = sum_j (1/count_j) * sum_i ce_ij / B

Strategy: columns are sharded over 8 cores; the HOST sorts all columns by
label first. Core c owns 1024 sorted columns whose same-label rows then
live in ONE contiguous row interval, padded to a 2048-row "window" (max
span ~1182 for this seed, so 1024 + 2*class always fits in LNW=1280).
Rows are passed to each core pre-split into et_win [D,2048] and et_main
[D,6144] (order of rows is irrelevant for the sums), which keeps the
program SPMD-identical across cores while the actual window offset varies
per core. All embedding inputs are bf16 (host-converted): no fp32r
staging copies, half the DMA bytes, and the first matmul can start as
soon as its tile lands.

The scalar engine (ACT) is the bottleneck (every logit passes through
exp), so it runs ONLY the exps and lns: all column reductions (S_j
partials, the ce sums) are DVE reduce_sums over the bf16/f32 exp outputs,
and the exp->S accumulator readouts are gone.

Per 128-column tile (main groups first so ACT starts early):
  main rows:   logits matmul (bf16) -> PSUM; ACT exp(+l/temp) -> bf16
               dump; DVE reduce -> S partials.
  window rows: logits matmul + one-hot mask matmul (-4.5 -> -C in logit
               units) -> PSUM; ACT exp(l/temp - C*same) -> Pwin (f32);
               DVE reduce -> masked S_win partial; DVE reciprocal
               R = 1/Pwin; ACT Ln(m_j * R + 1) -> bf16 dump; DVE reduce
               -> A_j partials, m_j = S_j*e^(eps-C): equals ce+eps at
               same-label entries, ~1e-17 at different-label ones.
Host: numer_sum_j = A_j - eps*count_j - log1p(S_j e^(eps-l_jj)) (the
diagonal term, which carries its own +eps), then a tiny f64 reduction.
Out-of-window same-label terms do not exist; out-of-window Ln terms are
< 1e-13 and are dropped.
"""
import numpy as np
import ml_dtypes
from contextlib import ExitStack

import concourse.bacc as bacc
import concourse.tile as tile
from concourse import mybir
from concourse.bass_utils import run_bass_kernel_spmd

B, D = 8192, 128
NCLS = 100
NCORES = 8
COLS = B // NCORES            # 1024 columns per core
NCT = COLS // 128             # 8 col-tiles per core
WIN = 2048                    # window rows per core
MAIN = B - WIN                # 6144 main rows per core
GROUP = 2048                  # rows per PSUM group (4 banks)
NGM = MAIN // GROUP           # 3 main groups
NSUB = GROUP // 512
LNW = 1280                    # Ln/recip slice: true same-label block size

TEMP = 0.07
EPS = 0.25
SCALE = float(np.float32(1.0) / np.float32(TEMP))   # exp scale (fp32 value)
MASKVAL = -4.5                                      # bf16-exact additive mask
C_USED = 4.5 * SCALE                                # mask size in logit units
MCONST = float(np.exp(EPS - C_USED))                # e^(eps-C)

_cache = {}


def _patch_act_tables():
    """Steer Exp and Ln onto the one table set holding both, so Exp/Ln
    alternation doesn't thrash ACT_TABLE_LOADs. Set ids are indices into
    act_info.json, so keep dict length/order and just hide exp/ln
    elsewhere."""
    import concourse.hw_specs as hw_specs
    from concourse import mybir as _mb
    if getattr(bacc, "_act_tables_patched", False):
        return
    orig = hw_specs.get_activation_tables

    def steer(arch):
        t = orig(arch)
        exp, ln = (_mb.ActivationFunctionType.Exp, _mb.ActivationFunctionType.Ln)
        if "natural_log_exp_and_others" not in t:
            return t
        return {k: (fns if k == "natural_log_exp_and_others"
                    else fns - {exp, ln}) for k, fns in t.items()}

    bacc.get_activation_tables = steer
    bacc._act_tables_patched = True


def _build():
    dt = mybir.dt
    _patch_act_tables()
    nc = bacc.Bacc("TRN2", target_bir_lowering=False, debug=False,
                   num_devices=NCORES)
    et_main = nc.dram_tensor("et_main", [D, MAIN], dt.bfloat16,
                             kind="ExternalInput").ap()
    et_win = nc.dram_tensor("et_win", [D, WIN], dt.bfloat16,
                            kind="ExternalInput").ap()
    et_own = nc.dram_tensor("et_own", [D, COLS], dt.bfloat16,
                            kind="ExternalInput").ap()
    oh_win = nc.dram_tensor("oh_win", [NCLS, WIN], dt.bfloat16,
                            kind="ExternalInput").ap()
    ohn_own = nc.dram_tensor("ohn_own", [NCLS, COLS], dt.bfloat16,
                             kind="ExternalInput").ap()
    NSLOT = NGM + 1                                    # S slots per col-tile
    out = nc.dram_tensor("out", [128, NCT * NSLOT + NCT], dt.float32,
                         kind="ExternalOutput").ap()

    with tile.TileContext(nc) as tc:
        with ExitStack() as ctx:
            const_pool = ctx.enter_context(tc.tile_pool(name="consts", bufs=1))
            p_pool = ctx.enter_context(tc.tile_pool(name="pwin", bufs=2))
            r_pool = ctx.enter_context(tc.tile_pool(name="rbuf", bufs=2))
            d_pool = ctx.enter_context(tc.tile_pool(name="dump", bufs=2))
            dl_pool = ctx.enter_context(tc.tile_pool(name="dumpln", bufs=2))
            ps_pool = ctx.enter_context(
                tc.tile_pool(name="psum", bufs=2, space="PSUM"))

            # Direct bf16 DMAs, two queues in parallel, ordered by first
            # consumption: ct0 runs main groups 0..2 then the window, so
            # the sync queue carries et_own + even main chunks + oh_win
            # and the gpsimd queue odd main chunks + et_win + ohn_own.
            t_et_own = const_pool.tile([D, COLS], dt.bfloat16)
            t_et_main = const_pool.tile([D, MAIN], dt.bfloat16)
            t_et_win = const_pool.tile([D, WIN], dt.bfloat16)
            t_oh_win = const_pool.tile([NCLS, WIN], dt.bfloat16)
            t_ohn_own = const_pool.tile([NCLS, COLS], dt.bfloat16)
            CH = 1024
            nc.sync.dma_start(t_et_own[:], et_own[:])
            nc.gpsimd.dma_start(t_et_main[:, 0:CH], et_main[:, 0:CH])
            for i in range(1, MAIN // CH):
                q = nc.sync if i % 2 == 1 else nc.gpsimd
                q.dma_start(t_et_main[:, i * CH:(i + 1) * CH],
                            et_main[:, i * CH:(i + 1) * CH])
            nc.gpsimd.dma_start(t_et_win[:], et_win[:])
            nc.sync.dma_start(t_oh_win[:], oh_win[:])
            nc.gpsimd.dma_start(t_ohn_own[:], ohn_own[:])

            s_part = const_pool.tile([128, NCT * NSLOT], dt.float32)
            a_part = const_pool.tile([128, NCT], dt.float32)
            m_raw = const_pool.tile([128, NCT], dt.float32)
            m_sb = const_pool.tile([128, NCT], dt.float32)

            def emit_ln(ct, R):
                # ce-sum: Ln(m_j / Pwin + 1); DVE owns the column sum.
                dump = dl_pool.tile([128, LNW], dt.bfloat16, tag="dump2")
                nc.scalar.activation(
                    dump[:], R[:], mybir.ActivationFunctionType.Ln,
                    scale=m_sb[:, ct:ct + 1], bias=1.0)
                nc.vector.reduce_sum(a_part[:, ct:ct + 1], dump[:],
                                     axis=mybir.AxisListType.X)

            prev = None          # (ct, R) whose Ln is deferred one col-tile
            for ct in range(NCT):
                lhs_et = t_et_own[:, ct * 128:(ct + 1) * 128]
                lhs_oh = t_ohn_own[:, ct * 128:(ct + 1) * 128]

                # ---- main rows: unmasked, only feed S ----
                for g in range(NGM):
                    r0 = g * GROUP
                    ps = ps_pool.tile([128, GROUP], dt.float32, tag="ps")
                    for n in range(NSUB):
                        nc.tensor.matmul(
                            ps[:, n * 512:(n + 1) * 512], lhs_et,
                            t_et_main[:, r0 + n * 512:r0 + (n + 1) * 512],
                            start=True, stop=True)
                    dump = d_pool.tile([128, GROUP], dt.bfloat16, tag="dump")
                    slot = ct * NSLOT + g
                    nc.scalar.activation(
                        dump[:], ps[:], mybir.ActivationFunctionType.Exp,
                        scale=SCALE)
                    nc.vector.reduce_sum(s_part[:, slot:slot + 1], dump[:],
                                         axis=mybir.AxisListType.X)

                # ---- window rows: masked; feed S and the ce sum ----
                ps = ps_pool.tile([128, GROUP], dt.float32, tag="ps")
                for n in range(NSUB):
                    nc.tensor.matmul(
                        ps[:, n * 512:(n + 1) * 512], lhs_et,
                        t_et_win[:, n * 512:(n + 1) * 512],
                        start=True, stop=False)
                for n in range(NSUB):
                    nc.tensor.matmul(
                        ps[:, n * 512:(n + 1) * 512], lhs_oh,
                        t_oh_win[:, n * 512:(n + 1) * 512],
                        start=False, stop=True)
                # Pwin = exp(l - C*same); f32 because the DVE fast
                # reciprocal needs fp32 bit layout.
                P = p_pool.tile([128, WIN], dt.float32, tag="P")
                slot = ct * NSLOT + NGM
                nc.scalar.activation(
                    P[:], ps[:], mybir.ActivationFunctionType.Exp,
                    scale=SCALE)
                nc.vector.reduce_sum(s_part[:, slot:slot + 1], P[:],
                                     axis=mybir.AxisListType.X)
                # Same-label rows sit at window offset 0 (host layout), so
                # the numerator path only needs the first LNW rows.
                R = r_pool.tile([128, LNW], dt.float32, tag="R")
                nc.vector.reciprocal_approx_fast(out=R[:], in_=P[:, 0:LNW])

                nc.vector.reduce_sum(
                    m_raw[:, ct:ct + 1],
                    s_part[:, ct * NSLOT:(ct + 1) * NSLOT],
                    axis=mybir.AxisListType.X)
                nc.vector.tensor_scalar_mul(
                    m_sb[:, ct:ct + 1], m_raw[:, ct:ct + 1], MCONST)

                # Defer this tile's Ln so the ACT FIFO can run the next
                # tile's exps while the DVE S/m chain completes.
                if prev is not None:
                    emit_ln(*prev)
                prev = (ct, R)
            emit_ln(*prev)

            nc.sync.dma_start(out[:, 0:NCT * NSLOT], s_part[:])
            nc.sync.dma_start(out[:, NCT * NSLOT:], a_part[:])
    nc.compile()
    return nc


def _get_nc():
    if "nc" not in _cache:
        _cache["nc"] = _build()
    return _cache["nc"]


def _prepare(embeds, labels):
    embeds = np.ascontiguousarray(np.asarray(embeds, dtype=np.float32))
    labels_i = np.asarray(labels).astype(np.int64)
    assert embeds.shape == (B, D)

    # Sort columns (and rows -- it is the same axis) by label so each
    # core's same-label rows are contiguous.
    perm = np.argsort(labels_i, kind="stable")
    lab = labels_i[perm]
    emb = embeds[perm]

    et = np.ascontiguousarray(emb.T).astype(ml_dtypes.bfloat16)   # [D, B]
    oh = np.zeros((NCLS, B), dtype=ml_dtypes.bfloat16)
    oh[lab, np.arange(B)] = ml_dtypes.bfloat16(1.0)
    ohn = (oh.astype(np.float32) * np.float32(MASKVAL)).astype(ml_dtypes.bfloat16)

    # class start/end in sorted order
    starts = np.searchsorted(lab, np.arange(NCLS), side="left")
    ends = np.searchsorted(lab, np.arange(NCLS), side="right")

    in_maps = []

    for c in range(NCORES):
        lo, hi = c * COLS, (c + 1) * COLS
        r_lo = int(starts[lab[lo]])
        r_hi = int(ends[lab[hi - 1]])
        span = r_hi - r_lo
        assert span <= LNW, f"window overflow: {span}"
        fill = WIN - span
        after = np.arange(r_hi, min(B, r_hi + fill))
        need = fill - len(after)
        before = np.arange(r_lo - need, r_lo) if need > 0 else np.arange(0)
        win_rows = np.concatenate([np.arange(r_lo, r_hi), after, before])
        assert len(win_rows) == WIN
        main_mask = np.ones(B, dtype=bool)
        main_mask[win_rows] = False
        main_idx = np.nonzero(main_mask)[0]
        in_maps.append({
            "et_main": np.ascontiguousarray(et[:, main_idx]),
            "et_win": np.ascontiguousarray(et[:, win_rows]),
            "et_own": np.ascontiguousarray(et[:, lo:hi]),
            "oh_win": np.ascontiguousarray(oh[:, win_rows]),
            "ohn_own": np.ascontiguousarray(ohn[:, lo:hi]),
        })
    return in_maps, lab, emb


def _combine(results, lab, emb):
    NSLOT = NGM + 1
    S = np.empty(B, dtype=np.float64)
    A = np.empty(B, dtype=np.float64)
    for c in range(NCORES):
        o = results[c]["out"].astype(np.float64)
        s = o[:, :NCT * NSLOT].reshape(128, NCT, NSLOT).sum(-1)   # [p, ct]
        a = o[:, NCT * NSLOT:NCT * NSLOT + NCT]                   # [p, ct]
        S[c * COLS:(c + 1) * COLS] = s.T.reshape(-1)              # j = ct*128+p
        A[c * COLS:(c + 1) * COLS] = a.T.reshape(-1)

    counts = np.bincount(lab, minlength=NCLS)
    count_j = counts[lab].astype(np.float64) - 1.0
    l_jj = (emb.astype(np.float64) ** 2).sum(1) * SCALE
    # A_j = sum_{in_numer}(ce+eps) + (ce_jj+eps); u_jj = ce_jj + eps, so
    # numer = A_j - eps*count_j - u_jj.
    u_jj = np.log1p(S * np.exp(EPS - l_jj))
    numer = A - EPS * count_j - u_jj
    loss = (numer / count_j).sum() / B
    return np.asarray(loss, dtype=np.float32)


def kernel(embeds, labels):
    in_maps, lab, emb = _prepare(embeds, labels)
    nc = _get_nc()
    res = run_bass_kernel_spmd(nc, in_maps, list(range(NCORES)))
    return _combine(res.results, lab, emb)


# revision 3
# speedup vs baseline: 1.3501x; 1.3501x over previous
"""EpsSupInfoNCE loss on 8 Trainium2 NeuronCores.

Math (reference): logits = (E @ E.T)/temp;  same[i,j] = (label_i == label_j)
  S_j   = sum_i exp(logits[i,j]) * (1 - same[i,j])     (masked column sums)
  ce_ij = log(exp(-eps) + S_j * exp(-logits[i,j]))     for same-label i != j
  loss  = sum_j (1/count_j) * sum_i ce_ij / B

Strategy: columns are sharded over 8 cores; the HOST sorts all columns by
label first. Core c owns 1024 sorted columns whose same-label rows then
live in ONE contiguous row interval, padded to a 2048-row "window" (max
span ~1182 for this seed; LNW=1280 covers it). Rows are passed to each
core pre-split into et_win [D,2048] and et_main [D,6144] (order of rows
is irrelevant for the sums), which keeps the program SPMD-identical
across cores while the actual window offset varies per core. All
embedding inputs are bf16 (host-converted): no fp32r staging copies,
half the DMA bytes, and the first matmul starts as soon as its tile
lands. The scalar engine (ACT) is the bottleneck (every logit passes
through exp); its fused per-instruction accumulator (182ns readout) is
the cheapest column-sum by far (a DVE reduce of the same group costs
2.6us), so ACT keeps all S/A accumulation fused, and outputs stay f32
(bf16 ACT output measures ~20% slower).

Per 128-column tile (main groups first so ACT starts early; ct0's first
group is split 512+1536 so the very first EXP only waits on two tiny
DMAs):
  main rows:   logits matmul (bf16, 512-wide) -> PSUM; ACT exp(+l/temp)
               with fused accum_out -> S partials.
  window rows: logits matmul + one-hot mask matmul (-4.5 -> -C in logit
               units) -> PSUM; ACT exp(l/temp - C*same) -> Pwin, whose
               fused accum IS the masked S_win partial; DVE reciprocal
               R = 1/Pwin; ACT Ln(m_j * R + 1) with accum_out -> A_j
               partials, m_j = S_j*e^(eps-C): equals ce+eps at same-label
               entries, ~1e-17 at different-label ones.
Host: numer_sum_j = A_j - eps*count_j - log1p(S_j e^(eps-l_jj)) (the
diagonal term, which carries its own +eps), then a tiny f64 reduction.
Out-of-window same-label terms do not exist; out-of-window Ln terms are
< 1e-13 and are dropped.
"""
import numpy as np
import ml_dtypes
from contextlib import ExitStack

import concourse.bacc as bacc
import concourse.tile as tile
from concourse import mybir
from concourse.bass_utils import run_bass_kernel_spmd

B, D = 8192, 128
NCLS = 100
NCORES = 8
COLS = B // NCORES            # 1024 columns per core
NCT = COLS // 128             # 8 col-tiles per core
WIN = 2048                    # window rows per core
MAIN = B - WIN                # 6144 main rows per core
GROUP = 2048                  # rows per PSUM group (4 banks)
NGM = MAIN // GROUP           # 3 main groups
NSUB = GROUP // 512
LNW = 1280                    # Ln/recip slice: true same-label block size
FIRST = 512                   # ct0's tiny first chunk (early ACT start)

TEMP = 0.07
EPS = 0.25
SCALE = float(np.float32(1.0) / np.float32(TEMP))   # exp scale (fp32 value)
MASKVAL = -4.5                                      # bf16-exact additive mask
C_USED = 4.5 * SCALE                                # mask size in logit units
MCONST = float(np.exp(EPS - C_USED))                # e^(eps-C)

_cache = {}


def _patch_act_tables():
    """Steer Exp and Ln onto the one table set holding both, so Exp/Ln
    alternation doesn't thrash ACT_TABLE_LOADs. Set ids are indices into
    act_info.json, so keep dict length/order and just hide exp/ln
    elsewhere."""
    import concourse.hw_specs as hw_specs
    from concourse import mybir as _mb
    if getattr(bacc, "_act_tables_patched", False):
        return
    orig = hw_specs.get_activation_tables

    def steer(arch):
        t = orig(arch)
        exp, ln = (_mb.ActivationFunctionType.Exp, _mb.ActivationFunctionType.Ln)
        if "natural_log_exp_and_others" not in t:
            return t
        return {k: (fns if k == "natural_log_exp_and_others"
                    else fns - {exp, ln}) for k, fns in t.items()}

    bacc.get_activation_tables = steer
    bacc._act_tables_patched = True


def _build():
    dt = mybir.dt
    _patch_act_tables()
    nc = bacc.Bacc("TRN2", target_bir_lowering=False, debug=False,
                   num_devices=NCORES)
    et_main = nc.dram_tensor("et_main", [D, MAIN], dt.bfloat16,
                             kind="ExternalInput").ap()
    et_win = nc.dram_tensor("et_win", [D, WIN], dt.bfloat16,
                            kind="ExternalInput").ap()
    et_own = nc.dram_tensor("et_own", [D, COLS], dt.bfloat16,
                            kind="ExternalInput").ap()
    oh_win = nc.dram_tensor("oh_win", [NCLS, WIN], dt.bfloat16,
                            kind="ExternalInput").ap()
    ohn_own = nc.dram_tensor("ohn_own", [NCLS, COLS], dt.bfloat16,
                             kind="ExternalInput").ap()
    NSLOT = NGM + 1                 # S slots per col-tile (3 main + 1 win)
    NS = 1 + NCT * NSLOT            # col 0: ct0's extra FIRST-chunk slot
    out = nc.dram_tensor("out", [128, NS + NCT], dt.float32,
                         kind="ExternalOutput").ap()

    with tile.TileContext(nc) as tc:
        with ExitStack() as ctx:
            const_pool = ctx.enter_context(tc.tile_pool(name="consts", bufs=1))
            p_pool = ctx.enter_context(tc.tile_pool(name="pwin", bufs=2))
            r_pool = ctx.enter_context(tc.tile_pool(name="rbuf", bufs=2))
            d_pool = ctx.enter_context(tc.tile_pool(name="dump", bufs=2))
            ps_pool = ctx.enter_context(
                tc.tile_pool(name="psum", bufs=2, space="PSUM"))

            # Direct bf16 DMAs, two queues in parallel, ordered by first
            # consumption: ct0 runs main chunks first (the 512-col head
            # chunk needs only ~190KB of DMA), then the window.
            t_et_own = const_pool.tile([D, COLS], dt.bfloat16)
            t_et_main = const_pool.tile([D, MAIN], dt.bfloat16)
            t_et_win = const_pool.tile([D, WIN], dt.bfloat16)
            t_oh_win = const_pool.tile([NCLS, WIN], dt.bfloat16)
            t_ohn_own = const_pool.tile([NCLS, COLS], dt.bfloat16)
            nc.sync.dma_start(t_et_own[:, 0:128], et_own[:, 0:128])
            nc.sync.dma_start(t_et_main[:, 0:FIRST], et_main[:, 0:FIRST])
            nc.gpsimd.dma_start(t_et_own[:, 128:], et_own[:, 128:])
            nc.sync.dma_start(t_et_main[:, FIRST:GROUP],
                              et_main[:, FIRST:GROUP])
            nc.gpsimd.dma_start(t_et_main[:, GROUP:GROUP + 1536],
                                et_main[:, GROUP:GROUP + 1536])
            nc.sync.dma_start(t_et_main[:, GROUP + 1536:2 * GROUP + 1024],
                              et_main[:, GROUP + 1536:2 * GROUP + 1024])
            nc.gpsimd.dma_start(t_et_main[:, 2 * GROUP + 1024:MAIN],
                                et_main[:, 2 * GROUP + 1024:MAIN])
            nc.gpsimd.dma_start(t_et_win[:], et_win[:])
            nc.sync.dma_start(t_oh_win[:], oh_win[:])
            nc.gpsimd.dma_start(t_ohn_own[:], ohn_own[:])

            s_part = const_pool.tile([128, NS], dt.float32)
            a_part = const_pool.tile([128, NCT], dt.float32)
            m_raw = const_pool.tile([128, NCT], dt.float32)
            m_sb = const_pool.tile([128, NCT], dt.float32)

            def emit_ln(ct, R):
                # ce-sum: Ln(m_j / Pwin + 1), fused per-column accumulate.
                dump = d_pool.tile([128, LNW], dt.float32, tag="dump2")
                nc.scalar.activation(
                    dump[:], R[:], mybir.ActivationFunctionType.Ln,
                    scale=m_sb[:, ct:ct + 1], bias=1.0,
                    accum_out=a_part[:, ct:ct + 1])

            def main_group(lhs_et, rows_lo, width, slot):
                ps = ps_pool.tile([128, GROUP], dt.float32, tag="ps")
                for n in range(width // 512):
                    nc.tensor.matmul(
                        ps[:, n * 512:(n + 1) * 512], lhs_et,
                        t_et_main[:, rows_lo + n * 512:
                                  rows_lo + (n + 1) * 512],
                        start=True, stop=True)
                dump = d_pool.tile([128, GROUP], dt.float32, tag="dump")
                nc.scalar.activation(
                    dump[:, 0:width], ps[:, 0:width],
                    mybir.ActivationFunctionType.Exp,
                    scale=SCALE, accum_out=s_part[:, slot:slot + 1])

            prev = None          # (ct, R) whose Ln is deferred one col-tile
            for ct in range(NCT):
                lhs_et = t_et_own[:, ct * 128:(ct + 1) * 128]
                lhs_oh = t_ohn_own[:, ct * 128:(ct + 1) * 128]
                base = 1 + ct * NSLOT

                # ---- main rows: unmasked, only feed S ----
                if ct == 0:
                    # split g0 so the first EXP waits on ~190KB of DMA
                    main_group(lhs_et, 0, FIRST, 0)
                    main_group(lhs_et, FIRST, GROUP - FIRST, base + 0)
                else:
                    main_group(lhs_et, 0 * GROUP, GROUP, base + 0)
                main_group(lhs_et, 1 * GROUP, GROUP, base + 1)
                main_group(lhs_et, 2 * GROUP, GROUP, base + 2)

                # ---- window rows: masked; feed S and the ce sum ----
                ps = ps_pool.tile([128, GROUP], dt.float32, tag="ps")
                for n in range(NSUB):
                    nc.tensor.matmul(
                        ps[:, n * 512:(n + 1) * 512], lhs_et,
                        t_et_win[:, n * 512:(n + 1) * 512],
                        start=True, stop=False)
                for n in range(NSUB):
                    nc.tensor.matmul(
                        ps[:, n * 512:(n + 1) * 512], lhs_oh,
                        t_oh_win[:, n * 512:(n + 1) * 512],
                        start=False, stop=True)
                # Pwin = exp(l - C*same); its fused accum IS the masked
                # S_win. The Ln input comes from the DVE reciprocal.
                P = p_pool.tile([128, WIN], dt.float32, tag="P")
                slot = base + NGM
                nc.scalar.activation(
                    P[:], ps[:], mybir.ActivationFunctionType.Exp,
                    scale=SCALE, accum_out=s_part[:, slot:slot + 1])
                # Same-label rows sit at window offset 0 (host layout), so
                # the numerator path only needs the first LNW rows.
                R = r_pool.tile([128, LNW], dt.float32, tag="R")
                nc.vector.reciprocal_approx_fast(out=R[:], in_=P[:, 0:LNW])

                lo = 0 if ct == 0 else base     # ct0's m includes slot 0
                nc.vector.reduce_sum(
                    m_raw[:, ct:ct + 1], s_part[:, lo:base + NSLOT],
                    axis=mybir.AxisListType.X)
                nc.vector.tensor_scalar_mul(
                    m_sb[:, ct:ct + 1], m_raw[:, ct:ct + 1], MCONST)

                # Defer this tile's Ln so the ACT FIFO can run the next
                # tile's exps while the DVE S/m chain completes.
                if prev is not None:
                    emit_ln(*prev)
                prev = (ct, R)
            emit_ln(*prev)

            nc.sync.dma_start(out[:, 0:NS], s_part[:])
            nc.sync.dma_start(out[:, NS:], a_part[:])
    nc.compile()
    return nc


def _get_nc():
    if "nc" not in _cache:
        _cache["nc"] = _build()
    return _cache["nc"]


def _prepare(embeds, labels):
    embeds = np.ascontiguousarray(np.asarray(embeds, dtype=np.float32))
    labels_i = np.asarray(labels).astype(np.int64)
    assert embeds.shape == (B, D)

    # Sort columns (and rows -- it is the same axis) by label so each
    # core's same-label rows are contiguous.
    perm = np.argsort(labels_i, kind="stable")
    lab = labels_i[perm]
    emb = embeds[perm]

    et = np.ascontiguousarray(emb.T).astype(ml_dtypes.bfloat16)   # [D, B]
    oh = np.zeros((NCLS, B), dtype=ml_dtypes.bfloat16)
    oh[lab, np.arange(B)] = ml_dtypes.bfloat16(1.0)
    ohn = (oh.astype(np.float32) * np.float32(MASKVAL)).astype(ml_dtypes.bfloat16)

    # class start/end in sorted order
    starts = np.searchsorted(lab, np.arange(NCLS), side="left")
    ends = np.searchsorted(lab, np.arange(NCLS), side="right")

    in_maps = []

    for c in range(NCORES):
        lo, hi = c * COLS, (c + 1) * COLS
        r_lo = int(starts[lab[lo]])
        r_hi = int(ends[lab[hi - 1]])
        span = r_hi - r_lo
        assert span <= LNW, f"window overflow: {span}"
        fill = WIN - span
        after = np.arange(r_hi, min(B, r_hi + fill))
        need = fill - len(after)
        before = np.arange(r_lo - need, r_lo) if need > 0 else np.arange(0)
        win_rows = np.concatenate([np.arange(r_lo, r_hi), after, before])
        assert len(win_rows) == WIN
        main_mask = np.ones(B, dtype=bool)
        main_mask[win_rows] = False
        main_idx = np.nonzero(main_mask)[0]
        in_maps.append({
            "et_main": np.ascontiguousarray(et[:, main_idx]),
            "et_win": np.ascontiguousarray(et[:, win_rows]),
            "et_own": np.ascontiguousarray(et[:, lo:hi]),
            "oh_win": np.ascontiguousarray(oh[:, win_rows]),
            "ohn_own": np.ascontiguousarray(ohn[:, lo:hi]),
        })
    return in_maps, lab, emb


def _combine(results, lab, emb):
    NSLOT = NGM + 1
    NS = 1 + NCT * NSLOT
    S = np.empty(B, dtype=np.float64)
    A = np.empty(B, dtype=np.float64)
    for c in range(NCORES):
        o = results[c]["out"].astype(np.float64)
        s = o[:, 1:NS].reshape(128, NCT, NSLOT).sum(-1)           # [p, ct]
        a = o[:, NS:NS + NCT]                                     # [p, ct]
        S[c * COLS:(c + 1) * COLS] = s.T.reshape(-1)              # j = ct*128+p
        S[c * COLS:c * COLS + 128] += o[:, 0]                     # ct0 extra
        A[c * COLS:(c + 1) * COLS] = a.T.reshape(-1)

    counts = np.bincount(lab, minlength=NCLS)
    count_j = counts[lab].astype(np.float64) - 1.0
    l_jj = (emb.astype(np.float64) ** 2).sum(1) * SCALE
    # A_j = sum_{in_numer}(ce+eps) + (ce_jj+eps); u_jj = ce_jj + eps, so
    # numer = A_j - eps*count_j - u_jj.
    u_jj = np.log1p(S * np.exp(EPS - l_jj))
    numer = A - EPS * count_j - u_jj
    loss = (numer / count_j).sum() / B
    return np.asarray(loss, dtype=np.float32)


def kernel(embeds, labels):
    in_maps, lab, emb = _prepare(embeds, labels)
    nc = _get_nc()
    res = run_bass_kernel_spmd(nc, in_maps, list(range(NCORES)))
    return _combine(res.results, lab, emb)
